# revision 54
# baseline (speedup 1.0000x reference)
"""Causal self-attention (BS=4, SL=2048, NE=1024, NH=16) on 8 trn2 NeuronCores.

Sharding (uniform SPMD program on all 8 cores, no collectives):
  core c -> batch b = c//2, head-group g = c%2 (8 of 16 heads, 512 feats).
  Each core: QKV proj for its heads (full 2048 rows of its batch),
  causal attention for its 8 heads, then a row-parallel out projection:
  partial_out = y_local[2048, 512] @ Wo[g*512:(g+1)*512, :]  (full 1024
  cols, fp16). Host reassembles out[b] = partial[2b] + partial[2b+1] + bo.
  No cross-core communication -> no collective sync / launch-skew stalls.

Matmul operands in fp16 (full PE rate, fp32 PSUM accumulate).
Attention computed in S^T = K @ Q^T layout so that:
  - PV needs no transposes: Y^T[65,q] += [V|1]^T @ expS^T (row 64 = denom)
  - softmax normalization via gpsimd partition_broadcast of 1/denom
  - y stays SBUF-resident [feat, seq] and feeds out-proj directly.
"""

import sys

if "/opt/trn_rl_repo" not in sys.path:
    sys.path.insert(0, "/opt/trn_rl_repo")

import numpy as np

import concourse.bass as bass
import concourse.mybir as mybir
import concourse.tile as tile
from concourse import bacc
from concourse.bass_utils import run_bass_kernel_spmd

F32 = mybir.dt.float32
F16 = mybir.dt.float16

# problem dims (hardcoded per spec)
BS, SL, NE, NH = 4, 2048, 1024, 16
HD = 64
N_CORES = 8


def build_nc(sl=SL, ne=NE, nh=NH, repeat=1):
    """Build the per-core Bass program. All 8 cores run this identically."""
    H = nh // 2          # local heads per core
    F = H * HD           # local feats (q/k/v width per core)
    FG = F // 128        # feat groups of 128 (2 heads each)
    CH = ne // 128       # contraction chunks for the qkv projection
    OCH = F // 128       # contraction chunks for the out projection
    PANEL = 512          # q-panel width
    NP = sl // PANEL     # number of q panels
    NKB = sl // 128      # number of 128-row k blocks
    VW = H * 65          # V' width (65-stride per head: 64 V cols + ones)

    s_bufs = 2
    nc = bacc.Bacc("TRN2", target_bir_lowering=False, num_devices=N_CORES)

    x = nc.dram_tensor("x", [sl, ne], F16, kind="ExternalInput")
    wq = nc.dram_tensor("wq", [ne, F], F16, kind="ExternalInput")
    wk = nc.dram_tensor("wk", [ne, F], F16, kind="ExternalInput")
    wv = nc.dram_tensor("wv", [ne, F], F16, kind="ExternalInput")
    bq = nc.dram_tensor("bq", [F], F32, kind="ExternalInput")
    bk = nc.dram_tensor("bk", [F], F32, kind="ExternalInput")
    wo = nc.dram_tensor("wo", [F, ne], F16, kind="ExternalInput")
    out = nc.dram_tensor("out", [sl, ne], F16, kind="ExternalOutput")


    with tile.TileContext(nc) as tc:
        with (
            tc.tile_pool(name="consts", bufs=1) as consts,
            tc.tile_pool(name="xload", bufs=1) as xload,
            tc.tile_pool(name="xt", bufs=3) as xtp,
            tc.tile_pool(name="qt", bufs=2) as qtp,
            tc.tile_pool(name="persist", bufs=1) as persist,
            tc.tile_pool(name="es", bufs=8) as esp,
            tc.tile_pool(name="misc", bufs=2) as misc,
            tc.tile_pool(name="psum", bufs=1, space="PSUM") as psp,
        ):
            # ---- constants: ident/tri built on-chip (no DMA chain at
            # t=0); biases DMA'd between the weight loads ----
            ones = consts.tile([128, 128], F16)
            nc.gpsimd.memset(ones, 1.0)
            ident = consts.tile([128, 128], F16)
            # ident[i,j] = 1 where j - i == 0
            nc.gpsimd.affine_select(
                ident, ones, [[1, 128]], mybir.AluOpType.is_equal, 0.0,
                channel_multiplier=-1)
            tri = consts.tile([128, 128], F16)
            # tri[i,j] = 1 where j - i >= 0 (upper triangular)
            nc.gpsimd.affine_select(
                tri, ones, [[1, 128]], mybir.AluOpType.is_ge, 0.0,
                channel_multiplier=-1)
            bqt = consts.tile([128, FG], F32)
            bkt = consts.tile([128, FG], F32)

            # panel-0 x^T via PE transposes: one batched row DMA lands much
            # faster than serialized DMA-transposes and PE is idle at start
            xT0 = [xtp.tile([128, PANEL], F16, tag=f"xt{c}", name=f"xT0_{c}")
                   for c in range(CH)]
            x_t = [xload.tile([128, ne], F16, name=f"x_t{sub}")
                   for sub in range(4)]
            for sub in range(4):
                nc.sync.dma_start(
                    out=x_t[sub], in_=x[sub * 128:(sub + 1) * 128, :])
            for c in range(CH):
                ps_x = psp.tile([128, PANEL], F16, tag="s", bufs=2,
                                name="ps_x")
                for sub in range(4):
                    nc.tensor.transpose(
                        ps_x[:, sub * 128:(sub + 1) * 128],
                        x_t[sub][:, c * 128:(c + 1) * 128], ident)
                nc.vector.tensor_copy(xT0[c], ps_x)

            # ---- resident weights, one batched DMA per matrix ----
            # (8 chunk-tiles fused into [128, CH*F] to pay the ~625ns HWDGE
            # descriptor-gen overhead once instead of 8x)
            WQt = persist.tile([128, CH * F], F16, tag="wq", name="WQt")
            WKt = persist.tile([128, CH * F], F16, tag="wk", name="WKt")
            WVt = persist.tile([128, CH * F], F16, tag="wv", name="WVt")
            WOt = persist.tile([128, OCH * ne], F16, tag="wo", name="WOt")
            WQ = [WQt[:, c * F:(c + 1) * F] for c in range(CH)]
            WK = [WKt[:, c * F:(c + 1) * F] for c in range(CH)]
            WV = [WVt[:, c * F:(c + 1) * F] for c in range(CH)]
            WO = [WOt[:, c * ne:(c + 1) * ne] for c in range(OCH)]
            for h0 in (0, 1):
                nc.sync.dma_start(
                    out=WQt[:, h0 * 4 * F:(h0 + 1) * 4 * F].rearrange(
                        "p (c f) -> p c f", f=F),
                    in_=wq[h0 * 512:(h0 + 1) * 512, :].rearrange(
                        "(c p) f -> p c f", p=128))
            nc.sync.dma_start(out=bqt, in_=bq.rearrange("(g p) -> p g", p=128))
            for h0 in (0, 1):
                nc.sync.dma_start(
                    out=WKt[:, h0 * 4 * F:(h0 + 1) * 4 * F].rearrange(
                        "p (c f) -> p c f", f=F),
                    in_=wk[h0 * 512:(h0 + 1) * 512, :].rearrange(
                        "(c p) f -> p c f", p=128))
            nc.sync.dma_start(out=bkt, in_=bk.rearrange("(g p) -> p g", p=128))
            nc.sync.dma_start(
                out=WVt.rearrange("p (c f) -> p c f", f=F),
                in_=wv.rearrange("(c p) f -> p c f", p=128))

            # WO is not needed until the first out-proj (after attn(0)):
            # defer its load off the critical early-DMA window
            def load_wo():
                nc.sync.dma_start(
                    out=WOt.rearrange("p (c f) -> p c f", f=ne),
                    in_=wo.rearrange("(c p) f -> p c f", p=128))

            # x^T stripes for panels 1..NP-1 in one DMA-transpose each,
            # queued up-front so the DMA engine streams continuously
            xTS = [persist.tile([128, sl - PANEL], F16, tag=f"xts{c}",
                                name=f"xTS{c}") for c in range(CH)]
            for c in range(CH):
                nc.sync.dma_start_transpose(
                    out=xTS[c], in_=x[PANEL:, c * 128:(c + 1) * 128])

            # ---- persistent attention operands ----
            KT = [persist.tile([128, sl], F16, tag=f"kt{f}", name=f"KT{f}")
                  for f in range(FG)]
            VP = [persist.tile([128, VW], F16, tag=f"vp{k}", name=f"VP{k}")
                  for k in range(NKB)]
            # SBUF-resident attention output, [feat, seq] layout
            Y = [persist.tile([128, sl], F16, tag=f"y{f}", name=f"Y{f}")
                 for f in range(FG)]

            for rep in range(repeat):
              QTs = {}

              def emit_xT(p, rep=None):
                  off = (p - 1) * PANEL
                  return [xTS[c][:, off:off + PANEL] for c in range(CH)]

              def proj_pieces(p, xT):
                  QT = [qtp.tile([128, PANEL], F16, tag=f"qt{f}",
                                 name=f"QT{f}") for f in range(FG)]
                  QTs[p] = QT
                  pieces = []

                  def qk_piece(f, wtiles, dst_qt, bias):
                      ps_a = psp.tile([128, PANEL], F32, tag="acc", bufs=2,
                                      name="ps_a")
                      for c in range(CH):
                          nc.tensor.matmul(
                              ps_a, wtiles[c][:, f * 128:(f + 1) * 128],
                              xT[c], start=(c == 0), stop=(c == CH - 1))
                      dst = (QT[f] if dst_qt
                             else KT[f][:, p * PANEL:(p + 1) * PANEL])
                      nc.scalar.activation(
                          dst, ps_a, mybir.ActivationFunctionType.Identity,
                          bias=bias[:, f:f + 1])

                  def v_piece(sub):
                      kb = p * 4 + sub
                      ps_v = psp.tile([128, F], F32, tag="acc", bufs=2,
                                      name="ps_v")
                      for c in range(CH):
                          nc.tensor.matmul(
                              ps_v, xT[c][:, sub * 128:(sub + 1) * 128],
                              WV[c], start=(c == 0), stop=(c == CH - 1))
                      vp3 = VP[kb].rearrange("p (h e) -> p h e", e=65)
                      nc.vector.memset(vp3[:, :, 64:65], 1.0)
                      nc.vector.tensor_copy(
                          vp3[:, :, 0:64],
                          ps_v.rearrange("p (h d) -> p h d", d=64))

                  for f in range(FG):
                      pieces.append(
                          lambda f=f: qk_piece(f, WQ, True, bqt))
                      pieces.append(
                          lambda f=f: qk_piece(f, WK, False, bkt))
                  for sub in range(4):
                      pieces.append(lambda sub=sub: v_piece(sub))
                  return pieces

              def attn_pieces(p, last_norm_sink=None):
                  QT = QTs[p]

                  def head_piece(h, norm_sink=None):
                      f, row = h // 2, (h % 2) * 64
                      ps_y = psp.tile([65, PANEL], F32, tag="y", bufs=2,
                                      name="ps_y")
                      nkb_p = 4 * p + 4

                      def emit_pv(es, offs):
                          for kb, d, n, o in offs:
                              nc.tensor.matmul(
                                  ps_y[:, d:PANEL],
                                  VP[kb][:, h * 65:h * 65 + 65],
                                  es[:, o:o + n],
                                  start=(kb == 0), stop=(kb == nkb_p - 1))

                      # software-pipelined: PV(i) is emitted after S(i+1)
                      # so exp(i) hides fully under S(i+1) on the PE stream
                      pending_pv = None
                      for kb0 in range(0, nkb_p, 2):
                          ps_s = psp.tile([128, 2 * PANEL], F32, tag="s",
                                          bufs=s_bufs, name="ps_s")
                          es = esp.tile([128, 2 * PANEL], F16, tag="es",
                                        name="es")
                          offs = []
                          for j in (0, 1):
                              kb = kb0 + j
                              d = max(0, (kb - 4 * p) * 128)
                              offs.append((kb, d, PANEL - d, j * PANEL))
                              nc.tensor.matmul(
                                  ps_s[:, j * PANEL:j * PANEL + PANEL - d],
                                  KT[f][row:row + 64,
                                        kb * 128:(kb + 1) * 128],
                                  QT[f][row:row + 64, d:PANEL])
                          if kb0 >= 4 * p:
                              for kb, d, n, o in offs:
                                  nc.scalar.activation(
                                      es[:, o:o + n], ps_s[:, o:o + n],
                                      mybir.ActivationFunctionType.Exp)
                                  nc.vector.tensor_mul(
                                      es[:, o:o + 128], es[:, o:o + 128], tri)
                          else:
                              nc.scalar.activation(
                                  es, ps_s, mybir.ActivationFunctionType.Exp)
                          if pending_pv is not None:
                              emit_pv(*pending_pv)
                          pending_pv = (es, offs)
                      emit_pv(*pending_pv)
                      def norm():
                          recip = misc.tile([1, PANEL], F32, tag="recip",
                                            bufs=3, name="recip")
                          nc.vector.reciprocal(recip, ps_y[64:65, :])
                          bc = misc.tile([64, PANEL], F32, tag="bc", bufs=3,
                                         name="bc")
                          nc.gpsimd.partition_broadcast(bc, recip)
                          ysl = Y[f][row:row + 64,
                                     p * PANEL:(p + 1) * PANEL]
                          nc.vector.tensor_mul(ysl, ps_y[0:64, :], bc)

                      if norm_sink is None:
                          norm()
                      else:
                          # defer the final head's norm so tail filler
                          # pieces aren't queued behind it on DVE
                          norm_sink.append(norm)

                  return [lambda h=h: head_piece(
                              h, last_norm_sink if h == H - 1 else None)
                          for h in range(H)]

              def out_pieces(p, copy_eng="vector", split_dma=False,
                             mix_psum=False):
                  # two 512-col halves on the "acc" PSUM tag (idle during
                  # the attention epilogue) so out-proj never contends
                  # with attention's "s" PSUM buffers
                  o_ts = {}

                  def out_piece(sb4, half, eng):
                      sb = p * (PANEL // 128) + sb4
                      hs = slice(half * 512, (half + 1) * 512)
                      # final panel: attention "s" PSUM is retired, so
                      # alternate tags to double the hoistable depth
                      tag = ("s" if mix_psum and (sb4 * 2 + half) % 2
                             else "acc")
                      ps_o = psp.tile([128, 512], F32, tag=tag, bufs=2,
                                      name="ps_o")
                      for c in range(OCH):
                          nc.tensor.matmul(
                              ps_o, Y[c][:, sb * 128:(sb + 1) * 128],
                              WO[c][:, hs], start=(c == 0),
                              stop=(c == OCH - 1))
                      if half == 0:
                          o_ts[sb] = misc.tile([128, ne], F16, tag="o_t",
                                               bufs=8, name="o_t")
                      o_t = o_ts[sb]
                      if eng == "scalar" or (eng == "alt" and half):
                          nc.scalar.copy(o_t[:, hs], ps_o)
                      else:
                          nc.vector.tensor_copy(o_t[:, hs], ps_o)
                      if split_dma:
                          # per-half DMA: shortens the final exposed
                          # copy->DMA chain at program end
                          nc.sync.dma_start(
                              out=out[sb * 128:(sb + 1) * 128, hs],
                              in_=o_t[:, hs])
                      elif half == 1:
                          # one batched DMA per seq-block (halves the tail
                          # HWDGE descriptor-gen serialization)
                          nc.sync.dma_start(
                              out=out[sb * 128:(sb + 1) * 128, :], in_=o_t)

                  return [lambda e=None, s=s, h=h:
                          out_piece(s, h, e or copy_eng)
                          for s in range(PANEL // 128) for h in (0, 1)]

              def interleave(attn, pp):
                  # pace pp pieces against attention pieces so the
                  # scheduler can fill PE gaps during the exp-bound
                  # attention pipeline
                  na, npj = len(attn), len(pp)
                  ai = pi = 0
                  while ai < na or pi < npj:
                      if ai < na:
                          attn[ai](); ai += 1
                      for _ in range(2):
                          if pi < npj and pi * na <= ai * npj:
                              pp[pi](); pi += 1

              prev_attn = None
              pending_out = []
              for p in range(NP):
                  xT = xT0 if (p == 0 and rep == 0) else emit_xT(p)
                  pp = proj_pieces(p, xT)
                  if p == 1 and rep == 0:
                      pp = [load_wo] + pp
                  if prev_attn is None:
                      for pc in pp:
                          pc()
                  else:
                      interleave(prev_attn, pp)
                      QTs.pop(p - 1, None)
                      # hold out-proj(p-1) back as PE filler for the
                      # final attention panel, which has no proj left
                      pending_out.extend(out_pieces(p - 1))
                  last_norm = [] if p == NP - 1 else None
                  prev_attn = attn_pieces(p, last_norm_sink=last_norm)
              interleave(prev_attn, pending_out)
              for n in last_norm:
                  n()
              for pc in out_pieces(NP - 1, copy_eng="alt",
                                   mix_psum=True):
                  pc()

    nc.compile()
    return nc


def shard_inputs(x, mask, Wqkv, bqkv, Wo, bo, sl=SL, ne=NE, nh=NH):
    """Host-side sharding: returns in_maps for the 8 cores."""
    H = nh // 2
    F = H * HD
    scale = 1.0 / np.sqrt(HD)
    in_maps = []
    for c in range(N_CORES):
        b, g = c // 2, c % 2
        qc = slice(g * F, (g + 1) * F)
        kc = slice(ne + g * F, ne + (g + 1) * F)
        vc = slice(2 * ne + g * F, 2 * ne + (g + 1) * F)
        in_maps.append({
            "x": np.ascontiguousarray(x[b]).astype(np.float16),
            "wq": (np.ascontiguousarray(Wqkv[:, qc]) * scale).astype(np.float16),
            "wk": np.ascontiguousarray(Wqkv[:, kc]).astype(np.float16),
            "wv": np.ascontiguousarray(Wqkv[:, vc]).astype(np.float16),
            "bq": np.ascontiguousarray(bqkv[qc]) * scale,
            "bk": np.ascontiguousarray(bqkv[kc]),
            "wo": np.ascontiguousarray(Wo[g * F:(g + 1) * F, :]).astype(np.float16),
        })
    return in_maps


def unshard_output(results, bo_eff, sl=SL, ne=NE):
    out = np.empty((BS, sl, ne), dtype=np.float32)
    for b in range(BS):
        out[b] = (results[2 * b]["out"].astype(np.float32)
                  + results[2 * b + 1]["out"].astype(np.float32) + bo_eff)
    return out


_NC_CACHE = {}


def kernel(x, mask, Wqkv, bqkv, Wo, bo):
    x = np.asarray(x, dtype=np.float32)
    Wqkv = np.asarray(Wqkv, dtype=np.float32)
    bqkv = np.asarray(bqkv, dtype=np.float32)
    Wo = np.asarray(Wo, dtype=np.float32)
    bo = np.asarray(bo, dtype=np.float32)
    if "nc" not in _NC_CACHE:
        _NC_CACHE["nc"] = build_nc()
    nc = _NC_CACHE["nc"]
    in_maps = shard_inputs(x, mask, Wqkv, bqkv, Wo, bo)
    res = run_bass_kernel_spmd(nc, in_maps, list(range(N_CORES)))
    _NC_CACHE["last_res"] = res
    # v-bias folded on host: softmax rows sum to 1, so y = sm(S)(xWv) + bv
    # and out = y@Wo + bo = device_partials + (bv@Wo + bo)
    bo_eff = bo.astype(np.float64) + bqkv[2 * NE:].astype(np.float64) @ Wo.astype(np.float64)
    return unshard_output(res.results, bo_eff.astype(np.float32))


# revision 61
# speedup vs baseline: 1.0203x; 1.0203x over previous
"""Causal self-attention (BS=4, SL=2048, NE=1024, NH=16) on 8 trn2 NeuronCores.

Sharding (uniform SPMD program on all 8 cores, no collectives):
  core c -> batch b = c//2, head-group g = c%2 (8 of 16 heads, 512 feats).
  Each core: QKV proj for its heads (full 2048 rows of its batch),
  causal attention for its 8 heads, then a row-parallel out projection:
  partial_out = y_local[2048, 512] @ Wo[g*512:(g+1)*512, :]  (full 1024
  cols, fp16). Host reassembles out[b] = partial[2b] + partial[2b+1] + bo.
  No cross-core communication -> no collective sync / launch-skew stalls.

Matmul operands in fp16 (full PE rate, fp32 PSUM accumulate).
Attention computed in S^T = K @ Q^T layout so that:
  - PV needs no transposes: Y^T[65,q] += [V|1]^T @ expS^T (row 64 = denom)
  - softmax normalization via gpsimd partition_broadcast of 1/denom
  - y stays SBUF-resident [feat, seq] and feeds out-proj directly.
"""

import sys

if "/opt/trn_rl_repo" not in sys.path:
    sys.path.insert(0, "/opt/trn_rl_repo")

import numpy as np

import concourse.bass as bass
import concourse.mybir as mybir
import concourse.tile as tile
from concourse import bacc
from concourse.bass_utils import run_bass_kernel_spmd

F32 = mybir.dt.float32
F16 = mybir.dt.float16

# problem dims (hardcoded per spec)
BS, SL, NE, NH = 4, 2048, 1024, 16
HD = 64
N_CORES = 8


def build_nc(sl=SL, ne=NE, nh=NH, repeat=1):
    """Build the per-core Bass program. All 8 cores run this identically."""
    H = nh // 2          # local heads per core
    F = H * HD           # local feats (q/k/v width per core)
    FG = F // 128        # feat groups of 128 (2 heads each)
    CH = ne // 128       # contraction chunks for the qkv projection
    OCH = F // 128       # contraction chunks for the out projection
    PANEL = 512          # q-panel width
    NP = sl // PANEL     # number of q panels
    NKB = sl // 128      # number of 128-row k blocks
    VW = H * 65          # V' width (65-stride per head: 64 V cols + ones)

    s_bufs = 2
    nc = bacc.Bacc("TRN2", target_bir_lowering=False, num_devices=N_CORES)

    x = nc.dram_tensor("x", [sl, ne], F16, kind="ExternalInput")
    wq = nc.dram_tensor("wq", [ne, F], F16, kind="ExternalInput")
    wk = nc.dram_tensor("wk", [ne, F], F16, kind="ExternalInput")
    wv = nc.dram_tensor("wv", [ne, F], F16, kind="ExternalInput")
    bq = nc.dram_tensor("bq", [F], F32, kind="ExternalInput")
    bk = nc.dram_tensor("bk", [F], F32, kind="ExternalInput")
    wo = nc.dram_tensor("wo", [F, ne], F16, kind="ExternalInput")
    out = nc.dram_tensor("out", [sl, ne], F16, kind="ExternalOutput")


    with tile.TileContext(nc) as tc:
        with (
            tc.tile_pool(name="consts", bufs=1) as consts,
            tc.tile_pool(name="xload", bufs=1) as xload,
            tc.tile_pool(name="xt", bufs=3) as xtp,
            tc.tile_pool(name="qt", bufs=2) as qtp,
            tc.tile_pool(name="persist", bufs=1) as persist,
            tc.tile_pool(name="es", bufs=8) as esp,
            tc.tile_pool(name="misc", bufs=2) as misc,
            tc.tile_pool(name="psum", bufs=1, space="PSUM") as psp,
        ):
            # ---- constants: ident/tri built on-chip (no DMA chain at
            # t=0); biases DMA'd between the weight loads ----
            ones = consts.tile([128, 128], F16)
            nc.gpsimd.memset(ones, 1.0)
            ident = consts.tile([128, 128], F16)
            # ident[i,j] = 1 where j - i == 0
            nc.gpsimd.affine_select(
                ident, ones, [[1, 128]], mybir.AluOpType.is_equal, 0.0,
                channel_multiplier=-1)
            tri = consts.tile([128, 128], F16)
            # tri[i,j] = 1 where j - i >= 0 (upper triangular)
            nc.gpsimd.affine_select(
                tri, ones, [[1, 128]], mybir.AluOpType.is_ge, 0.0,
                channel_multiplier=-1)
            bqt = consts.tile([128, FG], F32)
            bkt = consts.tile([128, FG], F32)

            # panel-0 x^T via PE transposes: one batched row DMA lands much
            # faster than serialized DMA-transposes and PE is idle at start
            xT0 = [xtp.tile([128, PANEL], F16, tag=f"xt{c}", name=f"xT0_{c}")
                   for c in range(CH)]
            x_t = [xload.tile([128, ne], F16, name=f"x_t{sub}")
                   for sub in range(4)]
            for sub in range(4):
                nc.sync.dma_start(
                    out=x_t[sub], in_=x[sub * 128:(sub + 1) * 128, :])
            for c in range(CH):
                ps_x = psp.tile([128, PANEL], F16, tag="s", bufs=2,
                                name="ps_x")
                for sub in range(4):
                    nc.tensor.transpose(
                        ps_x[:, sub * 128:(sub + 1) * 128],
                        x_t[sub][:, c * 128:(c + 1) * 128], ident)
                nc.vector.tensor_copy(xT0[c], ps_x)

            # ---- resident weights, one batched DMA per matrix ----
            # (8 chunk-tiles fused into [128, CH*F] to pay the ~625ns HWDGE
            # descriptor-gen overhead once instead of 8x)
            WQt = persist.tile([128, CH * F], F16, tag="wq", name="WQt")
            WKt = persist.tile([128, CH * F], F16, tag="wk", name="WKt")
            WVt = persist.tile([128, CH * F], F16, tag="wv", name="WVt")
            WOt = persist.tile([128, OCH * ne], F16, tag="wo", name="WOt")
            WQ = [WQt[:, c * F:(c + 1) * F] for c in range(CH)]
            WK = [WKt[:, c * F:(c + 1) * F] for c in range(CH)]
            WV = [WVt[:, c * F:(c + 1) * F] for c in range(CH)]
            WO = [WOt[:, c * ne:(c + 1) * ne] for c in range(OCH)]
            for h0 in (0, 1):
                nc.sync.dma_start(
                    out=WQt[:, h0 * 4 * F:(h0 + 1) * 4 * F].rearrange(
                        "p (c f) -> p c f", f=F),
                    in_=wq[h0 * 512:(h0 + 1) * 512, :].rearrange(
                        "(c p) f -> p c f", p=128))
            nc.sync.dma_start(out=bqt, in_=bq.rearrange("(g p) -> p g", p=128))
            for h0 in (0, 1):
                nc.sync.dma_start(
                    out=WKt[:, h0 * 4 * F:(h0 + 1) * 4 * F].rearrange(
                        "p (c f) -> p c f", f=F),
                    in_=wk[h0 * 512:(h0 + 1) * 512, :].rearrange(
                        "(c p) f -> p c f", p=128))
            nc.sync.dma_start(out=bkt, in_=bk.rearrange("(g p) -> p g", p=128))
            nc.sync.dma_start(
                out=WVt.rearrange("p (c f) -> p c f", f=F),
                in_=wv.rearrange("(c p) f -> p c f", p=128))

            # WO is not needed until the first out-proj (after attn(0)):
            # defer its load off the critical early-DMA window
            def load_wo():
                nc.sync.dma_start(
                    out=WOt.rearrange("p (c f) -> p c f", f=ne),
                    in_=wo.rearrange("(c p) f -> p c f", p=128))

            # x^T stripes for panels 1..NP-1 in one DMA-transpose each,
            # queued up-front so the DMA engine streams continuously
            xTS = [persist.tile([128, sl - PANEL], F16, tag=f"xts{c}",
                                name=f"xTS{c}") for c in range(CH)]
            for c in range(CH):
                nc.sync.dma_start_transpose(
                    out=xTS[c], in_=x[PANEL:, c * 128:(c + 1) * 128])

            # ---- persistent attention operands ----
            KT = [persist.tile([128, sl], F16, tag=f"kt{f}", name=f"KT{f}")
                  for f in range(FG)]
            VP = [persist.tile([128, VW], F16, tag=f"vp{k}", name=f"VP{k}")
                  for k in range(NKB)]
            # SBUF-resident attention output, [feat, seq] layout
            Y = [persist.tile([128, sl], F16, tag=f"y{f}", name=f"Y{f}")
                 for f in range(FG)]

            for rep in range(repeat):
              QTs = {}

              def emit_xT(p, rep=None):
                  off = (p - 1) * PANEL
                  return [xTS[c][:, off:off + PANEL] for c in range(CH)]

              def proj_pieces(p, xT):
                  QT = [qtp.tile([128, PANEL], F16, tag=f"qt{f}",
                                 name=f"QT{f}") for f in range(FG)]
                  QTs[p] = QT
                  pieces = []

                  def qk_piece(f, wtiles, dst_qt, bias):
                      ps_a = psp.tile([128, PANEL], F32, tag="acc", bufs=2,
                                      name="ps_a")
                      for c in range(CH):
                          nc.tensor.matmul(
                              ps_a, wtiles[c][:, f * 128:(f + 1) * 128],
                              xT[c], start=(c == 0), stop=(c == CH - 1))
                      dst = (QT[f] if dst_qt
                             else KT[f][:, p * PANEL:(p + 1) * PANEL])
                      nc.scalar.activation(
                          dst, ps_a, mybir.ActivationFunctionType.Identity,
                          bias=bias[:, f:f + 1])

                  def v_piece(sub):
                      kb = p * 4 + sub
                      ps_v = psp.tile([128, F], F32, tag="acc", bufs=2,
                                      name="ps_v")
                      for c in range(CH):
                          nc.tensor.matmul(
                              ps_v, xT[c][:, sub * 128:(sub + 1) * 128],
                              WV[c], start=(c == 0), stop=(c == CH - 1))
                      vp3 = VP[kb].rearrange("p (h e) -> p h e", e=65)
                      nc.vector.memset(vp3[:, :, 64:65], 1.0)
                      nc.vector.tensor_copy(
                          vp3[:, :, 0:64],
                          ps_v.rearrange("p (h d) -> p h d", d=64))

                  for f in range(FG):
                      pieces.append(
                          lambda f=f: qk_piece(f, WQ, True, bqt))
                      pieces.append(
                          lambda f=f: qk_piece(f, WK, False, bkt))
                  for sub in range(4):
                      pieces.append(lambda sub=sub: v_piece(sub))
                  return pieces

              def attn_pieces(p, last_norm_sink=None):
                  QT = QTs[p]

                  def head_piece(h, norm_sink=None):
                      f, row = h // 2, (h % 2) * 64
                      ps_y = psp.tile([65, PANEL], F32, tag="y", bufs=2,
                                      name="ps_y")
                      nkb_p = 4 * p + 4

                      def emit_pv(es, offs):
                          for kb, d, n, o in offs:
                              nc.tensor.matmul(
                                  ps_y[:, d:PANEL],
                                  VP[kb][:, h * 65:h * 65 + 65],
                                  es[:, o:o + n],
                                  start=(kb == 0), stop=(kb == nkb_p - 1))

                      # software-pipelined: PV trails S by two pairs so
                      # exp hides fully under the S stream on PE
                      from collections import deque
                      pending_pv = deque()
                      for kb0 in range(0, nkb_p, 2):
                          ps_s = psp.tile([128, 2 * PANEL], F32, tag="s",
                                          bufs=s_bufs, name="ps_s")
                          es = esp.tile([128, 2 * PANEL], F16, tag="es",
                                        name="es")
                          offs = []
                          for j in (0, 1):
                              kb = kb0 + j
                              d = max(0, (kb - 4 * p) * 128)
                              offs.append((kb, d, PANEL - d, j * PANEL))
                              nc.tensor.matmul(
                                  ps_s[:, j * PANEL:j * PANEL + PANEL - d],
                                  KT[f][row:row + 64,
                                        kb * 128:(kb + 1) * 128],
                                  QT[f][row:row + 64, d:PANEL])
                          if kb0 >= 4 * p:
                              for kb, d, n, o in offs:
                                  nc.scalar.activation(
                                      es[:, o:o + n], ps_s[:, o:o + n],
                                      mybir.ActivationFunctionType.Exp)
                                  nc.vector.tensor_mul(
                                      es[:, o:o + 128], es[:, o:o + 128], tri)
                          else:
                              nc.scalar.activation(
                                  es, ps_s, mybir.ActivationFunctionType.Exp)
                          pending_pv.append((es, offs))
                          if len(pending_pv) > 2:
                              emit_pv(*pending_pv.popleft())
                      while pending_pv:
                          emit_pv(*pending_pv.popleft())
                      def norm():
                          recip = misc.tile([1, PANEL], F32, tag="recip",
                                            bufs=3, name="recip")
                          nc.vector.reciprocal(recip, ps_y[64:65, :])
                          bc = misc.tile([64, PANEL], F32, tag="bc", bufs=3,
                                         name="bc")
                          nc.gpsimd.partition_broadcast(bc, recip)
                          ysl = Y[f][row:row + 64,
                                     p * PANEL:(p + 1) * PANEL]
                          nc.vector.tensor_mul(ysl, ps_y[0:64, :], bc)

                      if norm_sink is None:
                          norm()
                      else:
                          # defer the final head's norm so tail filler
                          # pieces aren't queued behind it on DVE
                          norm_sink.append(norm)

                  return [lambda h=h: head_piece(
                              h, last_norm_sink if h == H - 1 else None)
                          for h in range(H)]

              def out_pieces(p, copy_eng="vector", split_dma=False,
                             mix_psum=False):
                  # two 512-col halves on the "acc" PSUM tag (idle during
                  # the attention epilogue) so out-proj never contends
                  # with attention's "s" PSUM buffers
                  o_ts = {}

                  def out_piece(sb4, half, eng):
                      sb = p * (PANEL // 128) + sb4
                      hs = slice(half * 512, (half + 1) * 512)
                      # final panel: attention "s" PSUM is retired, so
                      # alternate tags to double the hoistable depth
                      tag = ("s" if mix_psum and (sb4 * 2 + half) % 2
                             else "acc")
                      ps_o = psp.tile([128, 512], F32, tag=tag, bufs=2,
                                      name="ps_o")
                      for c in range(OCH):
                          nc.tensor.matmul(
                              ps_o, Y[c][:, sb * 128:(sb + 1) * 128],
                              WO[c][:, hs], start=(c == 0),
                              stop=(c == OCH - 1))
                      if half == 0:
                          o_ts[sb] = misc.tile([128, ne], F16, tag="o_t",
                                               bufs=8, name="o_t")
                      o_t = o_ts[sb]
                      if eng == "alt" and half and sb4 == PANEL // 128 - 1:
                          # very last piece: split the drain into parallel
                          # quarter-copies + small DMAs to shorten the
                          # exposed end-of-program chain
                          q0, q1 = slice(half * 512, half * 512 + 256), \
                              slice(half * 512 + 256, (half + 1) * 512)
                          nc.vector.tensor_copy(o_t[:, q0], ps_o[:, 0:256])
                          nc.scalar.copy(o_t[:, q1], ps_o[:, 256:512])
                          nc.sync.dma_start(
                              out=out[sb * 128:(sb + 1) * 128, q0],
                              in_=o_t[:, q0])
                          nc.sync.dma_start(
                              out=out[sb * 128:(sb + 1) * 128, q1],
                              in_=o_t[:, q1])
                          nc.sync.dma_start(
                              out=out[sb * 128:(sb + 1) * 128, 0:512],
                              in_=o_t[:, 0:512])
                          return
                      if eng == "scalar" or (eng == "alt" and half):
                          nc.scalar.copy(o_t[:, hs], ps_o)
                      else:
                          nc.vector.tensor_copy(o_t[:, hs], ps_o)
                      if split_dma:
                          # per-half DMA: shortens the final exposed
                          # copy->DMA chain at program end
                          nc.sync.dma_start(
                              out=out[sb * 128:(sb + 1) * 128, hs],
                              in_=o_t[:, hs])
                      elif half == 1:
                          # one batched DMA per seq-block (halves the tail
                          # HWDGE descriptor-gen serialization)
                          nc.sync.dma_start(
                              out=out[sb * 128:(sb + 1) * 128, :], in_=o_t)

                  return [lambda e=None, s=s, h=h:
                          out_piece(s, h, e or copy_eng)
                          for s in range(PANEL // 128) for h in (0, 1)]

              def interleave(attn, pp):
                  # pace pp pieces against attention pieces so the
                  # scheduler can fill PE gaps during the exp-bound
                  # attention pipeline
                  na, npj = len(attn), len(pp)
                  ai = pi = 0
                  while ai < na or pi < npj:
                      if ai < na:
                          attn[ai](); ai += 1
                      for _ in range(2):
                          if pi < npj and pi * na <= ai * npj:
                              pp[pi](); pi += 1

              prev_attn = None
              pending_out = []
              for p in range(NP):
                  xT = xT0 if (p == 0 and rep == 0) else emit_xT(p)
                  pp = proj_pieces(p, xT)
                  if p == 1 and rep == 0:
                      pp = [load_wo] + pp
                  if prev_attn is None:
                      for pc in pp:
                          pc()
                  else:
                      interleave(prev_attn, pp)
                      QTs.pop(p - 1, None)
                      # hold out-proj(p-1) back as PE filler for the
                      # final attention panel, which has no proj left
                      pending_out.extend(out_pieces(p - 1))
                  last_norm = [] if p == NP - 1 else None
                  prev_attn = attn_pieces(p, last_norm_sink=last_norm)
              interleave(prev_attn, pending_out)
              for n in last_norm:
                  n()
              for pc in out_pieces(NP - 1, copy_eng="alt",
                                   mix_psum=True):
                  pc()

    nc.compile()
    return nc


def shard_inputs(x, mask, Wqkv, bqkv, Wo, bo, sl=SL, ne=NE, nh=NH):
    """Host-side sharding: returns in_maps for the 8 cores."""
    H = nh // 2
    F = H * HD
    scale = 1.0 / np.sqrt(HD)
    in_maps = []
    for c in range(N_CORES):
        b, g = c // 2, c % 2
        qc = slice(g * F, (g + 1) * F)
        kc = slice(ne + g * F, ne + (g + 1) * F)
        vc = slice(2 * ne + g * F, 2 * ne + (g + 1) * F)
        in_maps.append({
            "x": np.ascontiguousarray(x[b]).astype(np.float16),
            "wq": (np.ascontiguousarray(Wqkv[:, qc]) * scale).astype(np.float16),
            "wk": np.ascontiguousarray(Wqkv[:, kc]).astype(np.float16),
            "wv": np.ascontiguousarray(Wqkv[:, vc]).astype(np.float16),
            "bq": np.ascontiguousarray(bqkv[qc]) * scale,
            "bk": np.ascontiguousarray(bqkv[kc]),
            "wo": np.ascontiguousarray(Wo[g * F:(g + 1) * F, :]).astype(np.float16),
        })
    return in_maps


def unshard_output(results, bo_eff, sl=SL, ne=NE):
    out = np.empty((BS, sl, ne), dtype=np.float32)
    for b in range(BS):
        out[b] = (results[2 * b]["out"].astype(np.float32)
                  + results[2 * b + 1]["out"].astype(np.float32) + bo_eff)
    return out


_NC_CACHE = {}


def kernel(x, mask, Wqkv, bqkv, Wo, bo):
    x = np.asarray(x, dtype=np.float32)
    Wqkv = np.asarray(Wqkv, dtype=np.float32)
    bqkv = np.asarray(bqkv, dtype=np.float32)
    Wo = np.asarray(Wo, dtype=np.float32)
    bo = np.asarray(bo, dtype=np.float32)
    if "nc" not in _NC_CACHE:
        _NC_CACHE["nc"] = build_nc()
    nc = _NC_CACHE["nc"]
    in_maps = shard_inputs(x, mask, Wqkv, bqkv, Wo, bo)
    res = run_bass_kernel_spmd(nc, in_maps, list(range(N_CORES)))
    _NC_CACHE["last_res"] = res
    # v-bias folded on host: softmax rows sum to 1, so y = sm(S)(xWv) + bv
    # and out = y@Wo + bo = device_partials + (bv@Wo + bo)
    bo_eff = bo.astype(np.float64) + bqkv[2 * NE:].astype(np.float64) @ Wo.astype(np.float64)
    return unshard_output(res.results, bo_eff.astype(np.float32))


# revision 67
# speedup vs baseline: 1.0240x; 1.0036x over previous
"""Causal self-attention (BS=4, SL=2048, NE=1024, NH=16) on 8 trn2 NeuronCores.

Sharding (uniform SPMD program on all 8 cores, no collectives):
  core c -> batch b = c//2, head-group g = c%2 (8 of 16 heads, 512 feats).
  Each core: QKV proj for its heads (full 2048 rows of its batch),
  causal attention for its 8 heads, then a row-parallel out projection:
  partial_out = y_local[2048, 512] @ Wo[g*512:(g+1)*512, :]  (full 1024
  cols, fp16). Host reassembles out[b] = partial[2b] + partial[2b+1] + bo.
  No cross-core communication -> no collective sync / launch-skew stalls.

Matmul operands in fp16 (full PE rate, fp32 PSUM accumulate).
Attention computed in S^T = K @ Q^T layout so that:
  - PV needs no transposes: Y^T[65,q] += [V|1]^T @ expS^T (row 64 = denom)
  - softmax normalization via gpsimd partition_broadcast of 1/denom
  - y stays SBUF-resident [feat, seq] and feeds out-proj directly.
"""

import sys

if "/opt/trn_rl_repo" not in sys.path:
    sys.path.insert(0, "/opt/trn_rl_repo")

import numpy as np

import concourse.bass as bass
import concourse.mybir as mybir
import concourse.tile as tile
from concourse import bacc
from concourse.bass_utils import run_bass_kernel_spmd

F32 = mybir.dt.float32
F16 = mybir.dt.float16

# problem dims (hardcoded per spec)
BS, SL, NE, NH = 4, 2048, 1024, 16
HD = 64
N_CORES = 8


def build_nc(sl=SL, ne=NE, nh=NH, repeat=1):
    """Build the per-core Bass program. All 8 cores run this identically."""
    H = nh // 2          # local heads per core
    F = H * HD           # local feats (q/k/v width per core)
    FG = F // 128        # feat groups of 128 (2 heads each)
    CH = ne // 128       # contraction chunks for the qkv projection
    OCH = F // 128       # contraction chunks for the out projection
    PANEL = 512          # q-panel width
    NP = sl // PANEL     # number of q panels
    NKB = sl // 128      # number of 128-row k blocks
    VW = H * 65          # V' width (65-stride per head: 64 V cols + ones)

    s_bufs = 2
    nc = bacc.Bacc("TRN2", target_bir_lowering=False, num_devices=N_CORES)

    x = nc.dram_tensor("x", [sl, ne], F16, kind="ExternalInput")
    wq = nc.dram_tensor("wq", [ne, F], F16, kind="ExternalInput")
    wk = nc.dram_tensor("wk", [ne, F], F16, kind="ExternalInput")
    wv = nc.dram_tensor("wv", [ne, F], F16, kind="ExternalInput")
    bq = nc.dram_tensor("bq", [F], F32, kind="ExternalInput")
    bk = nc.dram_tensor("bk", [F], F32, kind="ExternalInput")
    wo = nc.dram_tensor("wo", [F, ne], F16, kind="ExternalInput")
    out = nc.dram_tensor("out", [sl, ne], F16, kind="ExternalOutput")


    with tile.TileContext(nc) as tc:
        with (
            tc.tile_pool(name="consts", bufs=1) as consts,
            tc.tile_pool(name="xload", bufs=1) as xload,
            tc.tile_pool(name="xt", bufs=3) as xtp,
            tc.tile_pool(name="qt", bufs=2) as qtp,
            tc.tile_pool(name="persist", bufs=1) as persist,
            tc.tile_pool(name="es", bufs=8) as esp,
            tc.tile_pool(name="misc", bufs=2) as misc,
            tc.tile_pool(name="psum", bufs=1, space="PSUM") as psp,
        ):
            # ---- constants: ident/tri built on-chip (no DMA chain at
            # t=0); biases DMA'd between the weight loads ----
            ones = consts.tile([128, 128], F16)
            nc.gpsimd.memset(ones, 1.0)
            ident = consts.tile([128, 128], F16)
            # ident[i,j] = 1 where j - i == 0
            nc.gpsimd.affine_select(
                ident, ones, [[1, 128]], mybir.AluOpType.is_equal, 0.0,
                channel_multiplier=-1)
            tri = consts.tile([128, 128], F16)
            # tri[i,j] = 1 where j - i >= 0 (upper triangular)
            nc.gpsimd.affine_select(
                tri, ones, [[1, 128]], mybir.AluOpType.is_ge, 0.0,
                channel_multiplier=-1)
            bqt = consts.tile([128, FG], F32)
            bkt = consts.tile([128, FG], F32)

            # panel-0 x^T via PE transposes: one batched row DMA lands much
            # faster than serialized DMA-transposes and PE is idle at start
            xT0 = [xtp.tile([128, PANEL], F16, tag=f"xt{c}", name=f"xT0_{c}")
                   for c in range(CH)]
            x_t = [xload.tile([128, ne], F16, name=f"x_t{sub}")
                   for sub in range(4)]
            for sub in range(4):
                nc.sync.dma_start(
                    out=x_t[sub], in_=x[sub * 128:(sub + 1) * 128, :])
            for c in range(CH):
                ps_x = psp.tile([128, PANEL], F16, tag="s", bufs=2,
                                name="ps_x")
                for sub in range(4):
                    nc.tensor.transpose(
                        ps_x[:, sub * 128:(sub + 1) * 128],
                        x_t[sub][:, c * 128:(c + 1) * 128], ident)
                nc.vector.tensor_copy(xT0[c], ps_x)

            # ---- resident weights, one batched DMA per matrix ----
            # (8 chunk-tiles fused into [128, CH*F] to pay the ~625ns HWDGE
            # descriptor-gen overhead once instead of 8x)
            WQt = persist.tile([128, CH * F], F16, tag="wq", name="WQt")
            WKt = persist.tile([128, CH * F], F16, tag="wk", name="WKt")
            WVt = persist.tile([128, CH * F], F16, tag="wv", name="WVt")
            WOt = persist.tile([128, OCH * ne], F16, tag="wo", name="WOt")
            WQ = [WQt[:, c * F:(c + 1) * F] for c in range(CH)]
            WK = [WKt[:, c * F:(c + 1) * F] for c in range(CH)]
            WV = [WVt[:, c * F:(c + 1) * F] for c in range(CH)]
            WO = [WOt[:, c * ne:(c + 1) * ne] for c in range(OCH)]
            for h0 in (0, 1):
                nc.sync.dma_start(
                    out=WQt[:, h0 * 4 * F:(h0 + 1) * 4 * F].rearrange(
                        "p (c f) -> p c f", f=F),
                    in_=wq[h0 * 512:(h0 + 1) * 512, :].rearrange(
                        "(c p) f -> p c f", p=128))
            nc.sync.dma_start(out=bqt, in_=bq.rearrange("(g p) -> p g", p=128))
            for h0 in (0, 1):
                nc.sync.dma_start(
                    out=WKt[:, h0 * 4 * F:(h0 + 1) * 4 * F].rearrange(
                        "p (c f) -> p c f", f=F),
                    in_=wk[h0 * 512:(h0 + 1) * 512, :].rearrange(
                        "(c p) f -> p c f", p=128))
            nc.sync.dma_start(out=bkt, in_=bk.rearrange("(g p) -> p g", p=128))
            nc.sync.dma_start(
                out=WVt.rearrange("p (c f) -> p c f", f=F),
                in_=wv.rearrange("(c p) f -> p c f", p=128))

            # WO is not needed until the first out-proj (after attn(0)):
            # defer its load off the critical early-DMA window
            def load_wo():
                nc.sync.dma_start(
                    out=WOt.rearrange("p (c f) -> p c f", f=ne),
                    in_=wo.rearrange("(c p) f -> p c f", p=128))

            # x^T stripes for panels 1..NP-1 in one DMA-transpose each,
            # queued up-front so the DMA engine streams continuously
            xTS = [persist.tile([128, sl - PANEL], F16, tag=f"xts{c}",
                                name=f"xTS{c}") for c in range(CH)]
            for c in range(CH):
                nc.sync.dma_start_transpose(
                    out=xTS[c], in_=x[PANEL:, c * 128:(c + 1) * 128])

            # ---- persistent attention operands ----
            KT = [persist.tile([128, sl], F16, tag=f"kt{f}", name=f"KT{f}")
                  for f in range(FG)]
            VP = [persist.tile([128, VW], F16, tag=f"vp{k}", name=f"VP{k}")
                  for k in range(NKB)]
            # SBUF-resident attention output, [feat, seq] layout
            Y = [persist.tile([128, sl], F16, tag=f"y{f}", name=f"Y{f}")
                 for f in range(FG)]

            for rep in range(repeat):
              QTs = {}

              def emit_xT(p, rep=None):
                  off = (p - 1) * PANEL
                  return [xTS[c][:, off:off + PANEL] for c in range(CH)]

              def proj_pieces(p, xT):
                  QT = [qtp.tile([128, PANEL], F16, tag=f"qt{f}",
                                 name=f"QT{f}") for f in range(FG)]
                  QTs[p] = QT
                  pieces = []

                  def qk_piece(f, wtiles, dst_qt, bias):
                      ps_a = psp.tile([128, PANEL], F32, tag="acc", bufs=2,
                                      name="ps_a")
                      for c in range(CH):
                          nc.tensor.matmul(
                              ps_a, wtiles[c][:, f * 128:(f + 1) * 128],
                              xT[c], start=(c == 0), stop=(c == CH - 1))
                      dst = (QT[f] if dst_qt
                             else KT[f][:, p * PANEL:(p + 1) * PANEL])
                      nc.scalar.activation(
                          dst, ps_a, mybir.ActivationFunctionType.Identity,
                          bias=bias[:, f:f + 1])

                  def v_piece(sub):
                      kb = p * 4 + sub
                      ps_v = psp.tile([128, F], F32, tag="acc", bufs=2,
                                      name="ps_v")
                      for c in range(CH):
                          nc.tensor.matmul(
                              ps_v, xT[c][:, sub * 128:(sub + 1) * 128],
                              WV[c], start=(c == 0), stop=(c == CH - 1))
                      vp3 = VP[kb].rearrange("p (h e) -> p h e", e=65)
                      nc.vector.memset(vp3[:, :, 64:65], 1.0)
                      nc.vector.tensor_copy(
                          vp3[:, :, 0:64],
                          ps_v.rearrange("p (h d) -> p h d", d=64))

                  for f in range(FG):
                      pieces.append(
                          lambda f=f: qk_piece(f, WQ, True, bqt))
                      pieces.append(
                          lambda f=f: qk_piece(f, WK, False, bkt))
                  for sub in range(4):
                      pieces.append(lambda sub=sub: v_piece(sub))
                  return pieces

              def attn_pieces(p, last_norm_sink=None):
                  QT = QTs[p]

                  def head_piece(h, norm_sink=None):
                      f, row = h // 2, (h % 2) * 64
                      ps_y = psp.tile([65, PANEL], F32, tag="y", bufs=2,
                                      name="ps_y")
                      nkb_p = 4 * p + 4

                      def emit_pv(es, offs):
                          for kb, d, n, o in offs:
                              nc.tensor.matmul(
                                  ps_y[:, d:PANEL],
                                  VP[kb][:, h * 65:h * 65 + 65],
                                  es[:, o:o + n],
                                  start=(kb == 0), stop=(kb == nkb_p - 1))

                      # software-pipelined: PV trails S by two pairs so
                      # exp hides fully under the S stream on PE
                      from collections import deque
                      pending_pv = deque()
                      for kb0 in range(0, nkb_p, 2):
                          ps_s = psp.tile([128, 2 * PANEL], F32, tag="s",
                                          bufs=s_bufs, name="ps_s")
                          es = esp.tile([128, 2 * PANEL], F16, tag="es",
                                        name="es")
                          offs = []
                          for j in (0, 1):
                              kb = kb0 + j
                              d = max(0, (kb - 4 * p) * 128)
                              offs.append((kb, d, PANEL - d, j * PANEL))
                              nc.tensor.matmul(
                                  ps_s[:, j * PANEL:j * PANEL + PANEL - d],
                                  KT[f][row:row + 64,
                                        kb * 128:(kb + 1) * 128],
                                  QT[f][row:row + 64, d:PANEL])
                          if kb0 >= 4 * p:
                              for kb, d, n, o in offs:
                                  nc.scalar.activation(
                                      es[:, o:o + n], ps_s[:, o:o + n],
                                      mybir.ActivationFunctionType.Exp)
                                  nc.vector.tensor_mul(
                                      es[:, o:o + 128], es[:, o:o + 128], tri)
                          else:
                              nc.scalar.activation(
                                  es, ps_s, mybir.ActivationFunctionType.Exp)
                          pending_pv.append((es, offs))
                          if len(pending_pv) > 2:
                              emit_pv(*pending_pv.popleft())
                      while pending_pv:
                          emit_pv(*pending_pv.popleft())
                      def norm():
                          recip = misc.tile([1, PANEL], F32, tag="recip",
                                            bufs=3, name="recip")
                          nc.vector.reciprocal(recip, ps_y[64:65, :])
                          bc = misc.tile([64, PANEL], F32, tag="bc", bufs=3,
                                         name="bc")
                          nc.gpsimd.partition_broadcast(bc, recip)
                          ysl = Y[f][row:row + 64,
                                     p * PANEL:(p + 1) * PANEL]
                          nc.vector.tensor_mul(ysl, ps_y[0:64, :], bc)

                      if norm_sink is None:
                          norm()
                      else:
                          # defer the final head's norm so tail filler
                          # pieces aren't queued behind it on DVE
                          norm_sink.append(norm)

                  return [lambda h=h: head_piece(
                              h, last_norm_sink if h == H - 1 else None)
                          for h in range(H)]

              def out_pieces(p, copy_eng="vector", split_dma=False,
                             mix_psum=False):
                  # two 512-col halves on the "acc" PSUM tag (idle during
                  # the attention epilogue) so out-proj never contends
                  # with attention's "s" PSUM buffers
                  o_ts = {}

                  def out_piece(sb4, half, eng):
                      sb = p * (PANEL // 128) + sb4
                      hs = slice(half * 512, (half + 1) * 512)
                      # final panel: attention "s" PSUM is retired, so
                      # alternate tags to double the hoistable depth
                      tag = (["acc", "s", "y"][(sb4 * 2 + half) % 3]
                             if mix_psum else "acc")
                      ps_o = psp.tile([128, 512], F32, tag=tag, bufs=2,
                                      name="ps_o")
                      for c in range(OCH):
                          nc.tensor.matmul(
                              ps_o, Y[c][:, sb * 128:(sb + 1) * 128],
                              WO[c][:, hs], start=(c == 0),
                              stop=(c == OCH - 1))
                      if half == 0:
                          o_ts[sb] = misc.tile([128, ne], F16, tag="o_t",
                                               bufs=8, name="o_t")
                      o_t = o_ts[sb]
                      if eng == "alt" and half and sb4 == PANEL // 128 - 1:
                          # very last piece: split the drain into parallel
                          # quarter-copies + small DMAs to shorten the
                          # exposed end-of-program chain
                          q0, q1 = slice(half * 512, half * 512 + 256), \
                              slice(half * 512 + 256, (half + 1) * 512)
                          nc.vector.tensor_copy(o_t[:, q0], ps_o[:, 0:256])
                          nc.scalar.copy(o_t[:, q1], ps_o[:, 256:512])
                          nc.sync.dma_start(
                              out=out[sb * 128:(sb + 1) * 128, q0],
                              in_=o_t[:, q0])
                          nc.sync.dma_start(
                              out=out[sb * 128:(sb + 1) * 128, q1],
                              in_=o_t[:, q1])
                          nc.sync.dma_start(
                              out=out[sb * 128:(sb + 1) * 128, 0:512],
                              in_=o_t[:, 0:512])
                          return
                      if eng == "scalar" or (eng == "alt" and half):
                          nc.scalar.copy(o_t[:, hs], ps_o)
                      else:
                          nc.vector.tensor_copy(o_t[:, hs], ps_o)
                      if split_dma:
                          # per-half DMA: shortens the final exposed
                          # copy->DMA chain at program end
                          nc.sync.dma_start(
                              out=out[sb * 128:(sb + 1) * 128, hs],
                              in_=o_t[:, hs])
                      elif half == 1:
                          # one batched DMA per seq-block (halves the tail
                          # HWDGE descriptor-gen serialization)
                          nc.sync.dma_start(
                              out=out[sb * 128:(sb + 1) * 128, :], in_=o_t)

                  return [lambda e=None, s=s, h=h:
                          out_piece(s, h, e or copy_eng)
                          for s in range(PANEL // 128) for h in (0, 1)]

              def interleave(attn, pp):
                  # pace pp pieces against attention pieces so the
                  # scheduler can fill PE gaps during the exp-bound
                  # attention pipeline
                  na, npj = len(attn), len(pp)
                  ai = pi = 0
                  while ai < na or pi < npj:
                      if ai < na:
                          attn[ai](); ai += 1
                      for _ in range(2):
                          if pi < npj and pi * na <= ai * npj:
                              pp[pi](); pi += 1

              prev_attn = None
              pending_out = []
              for p in range(NP):
                  xT = xT0 if (p == 0 and rep == 0) else emit_xT(p)
                  pp = proj_pieces(p, xT)
                  if p == 1 and rep == 0:
                      pp = [load_wo] + pp
                  if prev_attn is None:
                      for pc in pp:
                          pc()
                  else:
                      interleave(prev_attn, pp)
                      QTs.pop(p - 1, None)
                      # hold out-proj(p-1) back as PE filler for the
                      # final attention panel, which has no proj left
                      pending_out.extend(out_pieces(p - 1))
                  last_norm = [] if p == NP - 1 else None
                  prev_attn = attn_pieces(p, last_norm_sink=last_norm)
              interleave(prev_attn, pending_out)
              for n in last_norm:
                  n()
              for pc in out_pieces(NP - 1, copy_eng="alt",
                                   mix_psum=True):
                  pc()

    nc.compile()
    return nc


def shard_inputs(x, mask, Wqkv, bqkv, Wo, bo, sl=SL, ne=NE, nh=NH):
    """Host-side sharding: returns in_maps for the 8 cores."""
    H = nh // 2
    F = H * HD
    scale = 1.0 / np.sqrt(HD)
    in_maps = []
    for c in range(N_CORES):
        b, g = c // 2, c % 2
        qc = slice(g * F, (g + 1) * F)
        kc = slice(ne + g * F, ne + (g + 1) * F)
        vc = slice(2 * ne + g * F, 2 * ne + (g + 1) * F)
        in_maps.append({
            "x": np.ascontiguousarray(x[b]).astype(np.float16),
            "wq": (np.ascontiguousarray(Wqkv[:, qc]) * scale).astype(np.float16),
            "wk": np.ascontiguousarray(Wqkv[:, kc]).astype(np.float16),
            "wv": np.ascontiguousarray(Wqkv[:, vc]).astype(np.float16),
            "bq": np.ascontiguousarray(bqkv[qc]) * scale,
            "bk": np.ascontiguousarray(bqkv[kc]),
            "wo": np.ascontiguousarray(Wo[g * F:(g + 1) * F, :]).astype(np.float16),
        })
    return in_maps


def unshard_output(results, bo_eff, sl=SL, ne=NE):
    out = np.empty((BS, sl, ne), dtype=np.float32)
    for b in range(BS):
        out[b] = (results[2 * b]["out"].astype(np.float32)
                  + results[2 * b + 1]["out"].astype(np.float32) + bo_eff)
    return out


_NC_CACHE = {}


def kernel(x, mask, Wqkv, bqkv, Wo, bo):
    x = np.asarray(x, dtype=np.float32)
    Wqkv = np.asarray(Wqkv, dtype=np.float32)
    bqkv = np.asarray(bqkv, dtype=np.float32)
    Wo = np.asarray(Wo, dtype=np.float32)
    bo = np.asarray(bo, dtype=np.float32)
    if "nc" not in _NC_CACHE:
        _NC_CACHE["nc"] = build_nc()
    nc = _NC_CACHE["nc"]
    in_maps = shard_inputs(x, mask, Wqkv, bqkv, Wo, bo)
    res = run_bass_kernel_spmd(nc, in_maps, list(range(N_CORES)))
    _NC_CACHE["last_res"] = res
    # v-bias folded on host: softmax rows sum to 1, so y = sm(S)(xWv) + bv
    # and out = y@Wo + bo = device_partials + (bv@Wo + bo)
    bo_eff = bo.astype(np.float64) + bqkv[2 * NE:].astype(np.float64) @ Wo.astype(np.float64)
    return unshard_output(res.results, bo_eff.astype(np.float32))
